# revision 8
# baseline (speedup 1.0000x reference)
"""Trainium2 Bass kernel for ComponentBasedHierarchicalAttention.

Strategy: data-parallel over batch (B=8, one batch element per NeuronCore).
Per core, with X = x[b]^T [C=4608, HW=576] channel-major resident in SBUF:

  A1: qT[k,n]  = Wq @ X           (channel-major q)
  A2: k[m,c]   = (X^T Wk^T)       (token-major k)        + bias via ones-row matmul
  B:  v[t,d]   = (X^T Wv^T)       (token-major v, spilled to DRAM scratch)
  C:  per component i:
        cqT[m,n]  = Wc_i @ q^T
        energy[n,c] = cq_i @ k     (token-major, softmax along free dim)
        attn = softmax(energy); attnT via PE transpose
  D:  per 128-channel block d:
        out_i^T[d,n] = v[:,d]^T-contraction with attnT_i   (channel-major, -> attn_outs)
        combined^T = gamma*(sum_i out_i^T) + X[d]; PE-transpose -> combined (token-major)

All matmuls run in float32r (fp32 bits, 11-bit-mantissa rounding inside the PE,
full 1 cycle/row throughput at N>=256).
"""

import sys

if "/opt/trn_rl_repo" not in sys.path:
    sys.path.insert(0, "/opt/trn_rl_repo")

import numpy as np

B, H, Wd, C = 8, 24, 24, 4608
HW = H * Wd          # 576
C8 = 576
P = 128
NT = [(0, 128), (128, 128), (256, 128), (384, 128), (512, 64)]   # 576 partition tiles
CT = C // P          # 36 channel tiles
NCH = [(0, 288), (288, 288)]                                     # 576 free-dim chunks
N_CORES = 8


def build_nc():
    import concourse.bass as bass
    import concourse.mybir as mybir
    import concourse.tile as tile
    from concourse import bacc
    from concourse.masks import make_identity

    f32 = mybir.dt.float32
    f32r = mybir.dt.float32r
    Act = mybir.ActivationFunctionType
    Alu = mybir.AluOpType
    AxisX = mybir.AxisListType.X

    nc = bacc.Bacc("TRN2", target_bir_lowering=False, debug=False,
                   num_devices=N_CORES)

    X_d = nc.dram_tensor("X", [C, HW], f32r, kind="ExternalInput").ap()
    WqT_d = nc.dram_tensor("WqT", [C, C8], f32r, kind="ExternalInput").ap()
    WkT_d = nc.dram_tensor("WkT", [C, C8], f32r, kind="ExternalInput").ap()
    WvT_d = nc.dram_tensor("WvT", [C, C], f32r, kind="ExternalInput").ap()
    WcT_d = nc.dram_tensor("WcT", [3, C8, C8], f32r, kind="ExternalInput").ap()
    bq_d = nc.dram_tensor("bq", [C8], f32, kind="ExternalInput").ap()
    bc_d = nc.dram_tensor("bc", [3, C8], f32, kind="ExternalInput").ap()
    bk_d = nc.dram_tensor("bk", [C8], f32r, kind="ExternalInput").ap()
    bv_d = nc.dram_tensor("bv", [C], f32r, kind="ExternalInput").ap()
    gam_d = nc.dram_tensor("gamma", [1], f32, kind="ExternalInput").ap()
    ones_d = nc.dram_tensor("ones", [P], f32r, kind="ExternalInput").ap()
    comb_d = nc.dram_tensor("combined_out", [HW, C], f32, kind="ExternalOutput").ap()
    aout_d = nc.dram_tensor("attn_outs_out", [3, C, HW], f32, kind="ExternalOutput").ap()

    with tile.TileContext(nc) as tc:
        with tc.tile_pool(name="const", bufs=1) as const, \
             tc.tile_pool(name="dram", bufs=1, space="DRAM") as dram:
            ident = const.tile([P, P], f32)
            make_identity(nc, ident)
            ones = const.tile([1, P], f32r)
            nc.sync.dma_start(out=ones, in_=ones_d.rearrange("(p n) -> p n", p=1))

            bq_sb = const.tile([P, 5], f32)
            nc.sync.dma_start(out=bq_sb[:, 0:4],
                              in_=bq_d[0:512].rearrange("(t p) -> p t", p=P))
            nc.sync.dma_start(out=bq_sb[0:64, 4:5],
                              in_=bq_d[512:576].rearrange("(p t) -> p t", t=1))
            bc_sb = const.tile([P, 3, 5], f32)
            for i in range(3):
                nc.sync.dma_start(out=bc_sb[:, i, 0:4],
                                  in_=bc_d[i, 0:512].rearrange("(t p) -> p t", p=P))
                nc.sync.dma_start(out=bc_sb[0:64, i, 4:5],
                                  in_=bc_d[i, 512:576].rearrange("(p t) -> p t", t=1))
            gam_sb = const.tile([P, 1], f32)
            nc.sync.dma_start(out=gam_sb,
                              in_=bass.AP(tensor=gam_d.tensor, offset=gam_d.offset,
                                          ap=[[0, P], [1, 1]]))

            X_sb = const.tile([P, CT, HW], f32r)
            nc.sync.dma_start(out=X_sb, in_=X_d.rearrange("(ct p) n -> p ct n", p=P))
            qT_sb = const.tile([P, 5, HW], f32r)
            k_sb = const.tile([P, 5, C8], f32r)
            attnT_sb = const.tile([P, 3, 5, HW], f32r)
            v_dram = dram.tile([HW, C], f32r)

            # ---- Phase A1: qT[k, n] = Wq @ X  (channel-major q) ----
            with tc.tile_pool(name="wq", bufs=2) as wqp, \
                 tc.tile_pool(name="psA", bufs=4, space="PSUM") as psA:
                for kt, (koff, ksz) in enumerate(NT):
                    wq_blk = wqp.tile([P, CT, P], f32r, tag="wq")
                    nc.sync.dma_start(
                        out=wq_blk[:, :, 0:ksz],
                        in_=WqT_d[:, koff:koff + ksz].rearrange("(ct p) k -> p ct k", p=P))
                    for (noff, nsz) in NCH:
                        ps = psA.tile([P, 288], f32, tag="qps")
                        for ct in range(CT):
                            nc.tensor.matmul(ps[0:ksz, :],
                                             lhsT=wq_blk[:, ct, 0:ksz],
                                             rhs=X_sb[:, ct, noff:noff + nsz],
                                             start=(ct == 0), stop=(ct == CT - 1))
                        nc.scalar.activation(qT_sb[0:ksz, kt, noff:noff + nsz],
                                             ps[0:ksz, :], Act.Identity,
                                             bias=bq_sb[0:ksz, kt:kt + 1])

            # ---- Phase A2: k[m, c] token-major ----
            with tc.tile_pool(name="wk", bufs=2) as wkp, \
                 tc.tile_pool(name="bkp", bufs=1) as bkp, \
                 tc.tile_pool(name="psA2", bufs=7, space="PSUM") as psA2:
                bk_sb = bkp.tile([1, C8], f32r)
                nc.sync.dma_start(out=bk_sb, in_=bk_d.rearrange("(p n) -> p n", p=1))
                for (coff, csz) in NCH:
                    pss = [psA2.tile([P, 288], f32, tag="kps", name=f"kps{coff}_{j}")
                           for j in range(5)]
                    for mt, (moff, msz) in enumerate(NT):
                        nc.tensor.matmul(pss[mt][0:msz, :], lhsT=ones[0:1, 0:msz],
                                         rhs=bk_sb[:, coff:coff + csz],
                                         start=True, stop=False)
                    for half in range(2):
                        wk_blk = wkp.tile([P, 18, 288], f32r, tag="wk")
                        nc.sync.dma_start(
                            out=wk_blk[:, :, 0:csz],
                            in_=WkT_d[half * 2304:(half + 1) * 2304, coff:coff + csz]
                                .rearrange("(ct p) k -> p ct k", p=P))
                        for mt, (moff, msz) in enumerate(NT):
                            for ct in range(18):
                                nc.tensor.matmul(
                                    pss[mt][0:msz, :],
                                    lhsT=X_sb[:, half * 18 + ct, moff:moff + msz],
                                    rhs=wk_blk[:, ct, 0:csz],
                                    start=False, stop=(half == 1 and ct == 17))
                    for mt, (moff, msz) in enumerate(NT):
                        nc.scalar.activation(k_sb[0:msz, mt, coff:coff + csz],
                                             pss[mt][0:msz, :], Act.Copy)

            # ---- Phase B: v[t, d] token-major, spilled to DRAM ----
            with tc.tile_pool(name="wv", bufs=2) as wvp, \
                 tc.tile_pool(name="bvp", bufs=1) as bvp, \
                 tc.tile_pool(name="vb", bufs=3) as vbp, \
                 tc.tile_pool(name="psB", bufs=7, space="PSUM") as psB:
                bv_sb = bvp.tile([1, C], f32r)
                nc.sync.dma_start(out=bv_sb, in_=bv_d.rearrange("(p n) -> p n", p=1))
                for dch in range(9):
                    doff = dch * 512
                    pss = [psB.tile([P, 512], f32, tag="vps", name=f"vps{dch}_{j}")
                           for j in range(5)]
                    for mt, (moff, msz) in enumerate(NT):
                        nc.tensor.matmul(pss[mt][0:msz, :], lhsT=ones[0:1, 0:msz],
                                         rhs=bv_sb[:, doff:doff + 512],
                                         start=True, stop=False)
                    for quart in range(4):
                        wv_blk = wvp.tile([P, 9, 512], f32r, tag="wv")
                        nc.sync.dma_start(
                            out=wv_blk,
                            in_=WvT_d[quart * 1152:(quart + 1) * 1152, doff:doff + 512]
                                .rearrange("(ct p) d -> p ct d", p=P))
                        for mt, (moff, msz) in enumerate(NT):
                            for ct in range(9):
                                nc.tensor.matmul(
                                    pss[mt][0:msz, :],
                                    lhsT=X_sb[:, quart * 9 + ct, moff:moff + msz],
                                    rhs=wv_blk[:, ct, :],
                                    start=False, stop=(quart == 3 and ct == 8))
                    for mt, (moff, msz) in enumerate(NT):
                        vb = vbp.tile([P, 512], f32r, tag="vb")
                        nc.vector.tensor_copy(vb[0:msz, :], pss[mt][0:msz, :])
                        nc.sync.dma_start(out=v_dram[moff:moff + msz, doff:doff + 512],
                                          in_=vb[0:msz, :])

            # ---- Phase C: components -> attnT ----
            with tc.tile_pool(name="wc", bufs=2) as wcp, \
                 tc.tile_pool(name="cq", bufs=2) as cqp, \
                 tc.tile_pool(name="at", bufs=3) as atp, \
                 tc.tile_pool(name="sm", bufs=8) as smp, \
                 tc.tile_pool(name="psC", bufs=2, space="PSUM") as psC, \
                 tc.tile_pool(name="psE", bufs=4, space="PSUM") as psE, \
                 tc.tile_pool(name="psT", bufs=2, space="PSUM") as psT:
                for i in range(3):
                    wc_blk = wcp.tile([P, 5, C8], f32r, tag="wc")
                    nc.sync.dma_start(
                        out=wc_blk[:, 0:4, :],
                        in_=WcT_d[i, 0:512, :].rearrange("(t p) m -> p t m", p=P))
                    nc.sync.dma_start(out=wc_blk[0:64, 4, :], in_=WcT_d[i, 512:576, :])
                    cqT = cqp.tile([P, 5, HW], f32r, tag="cq")
                    for mt, (moff, msz) in enumerate(NT):
                        for (noff, nsz) in NCH:
                            ps = psC.tile([P, 288], f32, tag="cqps")
                            for kt, (koff, ksz) in enumerate(NT):
                                nc.tensor.matmul(ps[0:msz, :],
                                                 lhsT=wc_blk[0:ksz, kt, moff:moff + msz],
                                                 rhs=qT_sb[0:ksz, kt, noff:noff + nsz],
                                                 start=(kt == 0), stop=(kt == 4))
                            nc.scalar.activation(cqT[0:msz, mt, noff:noff + nsz],
                                                 ps[0:msz, :], Act.Identity,
                                                 bias=bc_sb[0:msz, i, mt:mt + 1])
                    for nt, (noff, nsz) in enumerate(NT):
                        eps = []
                        for (coff, csz) in NCH:
                            ps = psE.tile([P, 288], f32, tag="eps",
                                          name=f"eps{i}_{nt}_{coff}")
                            for mt, (moff, msz) in enumerate(NT):
                                nc.tensor.matmul(ps[0:nsz, :],
                                                 lhsT=cqT[0:msz, mt, noff:noff + nsz],
                                                 rhs=k_sb[0:msz, mt, coff:coff + csz],
                                                 start=(mt == 0), stop=(mt == 4))
                            eps.append(ps)
                        nm0 = smp.tile([P, 1], f32, tag="nm0")
                        nm1 = smp.tile([P, 1], f32, tag="nm1")
                        nc.vector.tensor_reduce(nm0[0:nsz, :], eps[0][0:nsz, :],
                                                axis=AxisX, op=Alu.max, negate=True)
                        nc.vector.tensor_reduce(nm1[0:nsz, :], eps[1][0:nsz, :],
                                                axis=AxisX, op=Alu.max, negate=True)
                        nm = smp.tile([P, 1], f32, tag="nm")
                        nc.vector.tensor_tensor(nm[0:nsz, :], nm0[0:nsz, :],
                                                nm1[0:nsz, :], op=Alu.min)
                        att = atp.tile([P, HW], f32, tag="att")
                        s0 = smp.tile([P, 1], f32, tag="s0")
                        s1 = smp.tile([P, 1], f32, tag="s1")
                        nc.scalar.activation(att[0:nsz, 0:288], eps[0][0:nsz, :],
                                             Act.Exp, bias=nm[0:nsz, :],
                                             accum_out=s0[0:nsz, :])
                        nc.scalar.activation(att[0:nsz, 288:576], eps[1][0:nsz, :],
                                             Act.Exp, bias=nm[0:nsz, :],
                                             accum_out=s1[0:nsz, :])
                        ssum = smp.tile([P, 1], f32, tag="ss")
                        rec = smp.tile([P, 1], f32, tag="rec")
                        nc.vector.tensor_add(ssum[0:nsz, :], s0[0:nsz, :], s1[0:nsz, :])
                        nc.vector.reciprocal(rec[0:nsz, :], ssum[0:nsz, :])
                        nc.vector.tensor_scalar_mul(att[0:nsz, :], att[0:nsz, :],
                                                    rec[0:nsz, 0:1])
                        for ctb, (coff, csz) in enumerate(NT):
                            pt = psT.tile([P, P], f32, tag="pt")
                            nc.tensor.transpose(pt[0:csz, 0:nsz],
                                                att[0:nsz, coff:coff + csz],
                                                ident[0:nsz, 0:nsz])
                            nc.vector.tensor_copy(
                                attnT_sb[0:csz, i, ctb, noff:noff + nsz],
                                pt[0:csz, 0:nsz])

            # ---- Phase D: out_i^T, combined ----
            with tc.tile_pool(name="vblk", bufs=3) as vbkp, \
                 tc.tile_pool(name="os", bufs=2) as osp, \
                 tc.tile_pool(name="stg", bufs=2) as stgp, \
                 tc.tile_pool(name="psD", bufs=4, space="PSUM") as psD, \
                 tc.tile_pool(name="psDT", bufs=2, space="PSUM") as psDT:
                stage = {}
                for dt in range(CT):
                    doff = dt * P
                    vb = vbkp.tile([P, 5, P], f32r, tag="vblk")
                    nc.sync.dma_start(
                        out=vb[:, 0:4, :],
                        in_=v_dram[0:512, doff:doff + P].rearrange("(t p) d -> p t d", p=P))
                    nc.sync.dma_start(out=vb[0:64, 4, :],
                                      in_=v_dram[512:576, doff:doff + P])
                    outs = []
                    for i in range(3):
                        ot = osp.tile([P, HW], f32, tag=f"os{i}")
                        for (noff, nsz) in NCH:
                            ps = psD.tile([P, 288], f32, tag="dps")
                            for tt, (toff, tsz) in enumerate(NT):
                                nc.tensor.matmul(
                                    ps[:, :], lhsT=vb[0:tsz, tt, :],
                                    rhs=attnT_sb[0:tsz, i, tt, noff:noff + nsz],
                                    start=(tt == 0), stop=(tt == 4))
                            nc.scalar.activation(ot[:, noff:noff + nsz], ps[:, :],
                                                 Act.Copy)
                        nc.sync.dma_start(out=aout_d[i, doff:doff + P, :], in_=ot)
                        outs.append(ot)
                    u = osp.tile([P, HW], f32, tag="u")
                    nc.gpsimd.tensor_add(u, outs[0], outs[1])
                    u2 = osp.tile([P, HW], f32, tag="u2")
                    nc.gpsimd.tensor_add(u2, u, outs[2])
                    cmb = osp.tile([P, HW], f32, tag="cmb")
                    nc.vector.scalar_tensor_tensor(
                        cmb, u2, gam_sb[:, 0:1],
                        X_sb[:, dt, :].bitcast(mybir.dt.float32),
                        op0=Alu.mult, op1=Alu.add)
                    grp = dt % 4
                    if grp == 0:
                        stage = {nt: stgp.tile([P, 512], f32, tag=f"st{nt}",
                                               name=f"st{nt}_{dt}")
                                 for nt in range(5)}
                    for nt, (noff, nsz) in enumerate(NT):
                        pt = psDT.tile([P, P], f32, tag="pdt")
                        nc.tensor.transpose(pt[0:nsz, 0:P], cmb[:, noff:noff + nsz],
                                            ident[:, :])
                        nc.vector.tensor_copy(stage[nt][0:nsz, grp * P:(grp + 1) * P],
                                              pt[0:nsz, 0:P])
                    if grp == 3:
                        for nt, (noff, nsz) in enumerate(NT):
                            nc.sync.dma_start(
                                out=comb_d[noff:noff + nsz, (dt - 3) * P:(dt + 1) * P],
                                in_=stage[nt][0:nsz, :])

    nc.compile()
    return nc


def run_spmd(nc, in_maps, time_iters=0):
    """Execute on the 8 axon cores via PJRT (no donation so the compiled fn can
    be re-invoked on device-resident buffers for timing)."""
    import jax
    import concourse.mybir as mybir
    from concourse import bass2jax
    from jax.sharding import Mesh, NamedSharding, PartitionSpec
    from jax.experimental.shard_map import shard_map

    bass2jax.install_neuronx_cc_hook()
    partition_name = nc.partition_id_tensor.name if nc.partition_id_tensor else None
    in_names, out_names, out_avals = [], [], []
    for alloc in nc.m.functions[0].allocations:
        if not isinstance(alloc, mybir.MemoryLocationSet):
            continue
        name = alloc.memorylocations[0].name
        if alloc.kind == "ExternalInput":
            if name != partition_name:
                in_names.append(name)
        elif alloc.kind == "ExternalOutput":
            out_names.append(name)
            out_avals.append(jax.core.ShapedArray(tuple(alloc.tensor_shape),
                                                  mybir.dt.np(alloc.dtype)))
    n_params = len(in_names)
    all_in = in_names + out_names
    if partition_name is not None:
        all_in = all_in + [partition_name]

    def _body(*args):
        operands = list(args)
        if partition_name is not None:
            operands.append(bass2jax.partition_id_tensor())
        return tuple(bass2jax._bass_exec_p.bind(
            *operands, out_avals=tuple(out_avals), in_names=tuple(all_in),
            out_names=tuple(out_names), lowering_input_output_aliases=(),
            sim_require_finite=True, sim_require_nnan=True, nc=nc))

    devices = jax.devices()[:N_CORES]
    mesh = Mesh(np.asarray(devices), ("core",))
    nspec = (PartitionSpec("core"),)
    sharded = jax.jit(shard_map(_body, mesh=mesh,
                                in_specs=nspec * (n_params + len(out_names)),
                                out_specs=nspec * len(out_names), check_rep=False),
                      keep_unused=True)
    sh = NamedSharding(mesh, PartitionSpec("core"))
    dev_in = [jax.device_put(
        np.concatenate([np.asarray(in_maps[c][nm]) for c in range(N_CORES)], axis=0),
        sh) for nm in in_names]
    dev_zero = [jax.device_put(
        np.zeros((N_CORES * a.shape[0], *a.shape[1:]), a.dtype), sh)
        for a in out_avals]
    out_arrs = sharded(*dev_in, *dev_zero)
    jax.block_until_ready(out_arrs)
    per_iter_ns = None
    if time_iters:
        import time
        t0 = time.perf_counter()
        for _ in range(time_iters):
            out_arrs = sharded(*dev_in, *dev_zero)
        jax.block_until_ready(out_arrs)
        per_iter_ns = (time.perf_counter() - t0) / time_iters * 1e9
    results = [{name: np.asarray(out_arrs[i]).reshape(N_CORES, *out_avals[i].shape)[c]
                for i, name in enumerate(out_names)} for c in range(N_CORES)]
    return results, per_iter_ns


def make_in_maps(x, Wq, bq, Wk, bk, Wv, bv, Wc, bc, gamma):
    f = lambda a: np.ascontiguousarray(np.asarray(a, dtype=np.float32))
    WqT = f(np.asarray(Wq).T)
    WkT = f(np.asarray(Wk).T)
    WvT = f(np.asarray(Wv).T)
    WcT = f(np.transpose(np.asarray(Wc), (0, 2, 1)))
    shared = {"WqT": WqT, "WkT": WkT, "WvT": WvT, "WcT": WcT,
              "bq": f(bq), "bk": f(bk), "bv": f(bv), "bc": f(bc),
              "gamma": f(gamma), "ones": np.ones(P, dtype=np.float32)}
    in_maps = []
    for b in range(B):
        Xb = f(np.asarray(x[b]).reshape(HW, C).T)
        in_maps.append({"X": Xb, **shared})
    return in_maps


def assemble(results):
    combined = np.stack([results[b]["combined_out"].reshape(H, Wd, C)
                         for b in range(B)], axis=0)
    attn_outs = np.stack([results[b]["attn_outs_out"].reshape(3, C, H, Wd)
                          for b in range(B)], axis=1)
    return combined, attn_outs


def kernel(x, Wq, bq, Wk, bk, Wv, bv, Wc, bc, gamma):
    nc = build_nc()
    in_maps = make_in_maps(x, Wq, bq, Wk, bk, Wv, bv, Wc, bc, gamma)
    results, _ = run_spmd(nc, in_maps)
    return assemble(results)


# revision 12
# speedup vs baseline: 2.6243x; 2.6243x over previous
"""Trainium2 Bass kernel for ComponentBasedHierarchicalAttention.

Strategy: data-parallel over batch (B=8, one batch element per NeuronCore).
All matmuls in bf16 (measured ~6x faster than fp32r on this silicon). The
softmax-critical chain (q, k, cq, energy) uses split-bf16: operands are
represented as hi+lo bf16 pairs and each matmul computes the 3-term expansion
hi@hi + hi@lo + lo@hi with fp32 PSUM accumulation (~2^-17 operand precision),
because the energy logits have std ~21 and softmax amplifies absolute logit
error. The v projection and attn@v run in plain bf16 (linear error paths).

Per core, X = x[b]^T [C=4608, HW=576] channel-major:
  A1: qT[k,n] = Wq @ X          (channel-major q, split-bf16, +bq via ACT bias)
  A2: k[m,c]  = (X^T Wk^T)      (token-major k, split-bf16, +bk via ones-row MM)
  B:  v[t,d]  = (X^T Wv^T)      (token-major v, bf16, resident in SBUF)
  C:  per component i: cqT = Wc_i @ q^T; energy = cq_i @ k (token-major);
      softmax along free dim; attn -> attnT via PE transpose
  D:  per 128-channel block: out_i^T[d,n] (channel-major -> attn_outs);
      combined^T = gamma*sum_i out_i^T + X; PE transpose -> combined
"""

import sys

if "/opt/trn_rl_repo" not in sys.path:
    sys.path.insert(0, "/opt/trn_rl_repo")

import numpy as np

B, H, Wd, C = 8, 24, 24, 4608
HW = H * Wd          # 576
C8 = 576
P = 128
NT = [(0, 128), (128, 128), (256, 128), (384, 128), (512, 64)]   # 576 partition tiles
CT = C // P          # 36 channel tiles
NCH = [(0, 288), (288, 288)]                                     # 576 free-dim chunks
N_CORES = 8


def build_nc():
    import concourse.bass as bass
    import concourse.mybir as mybir
    import concourse.tile as tile
    from concourse import bacc

    f32 = mybir.dt.float32
    bf16 = mybir.dt.bfloat16
    Act = mybir.ActivationFunctionType
    Alu = mybir.AluOpType
    AxisX = mybir.AxisListType.X

    nc = bacc.Bacc("TRN2", target_bir_lowering=False, debug=False,
                   num_devices=N_CORES)

    X32_d = nc.dram_tensor("X32", [C, HW], f32, kind="ExternalInput").ap()
    Xh_d = nc.dram_tensor("Xh", [C, HW], bf16, kind="ExternalInput").ap()
    Xl_d = nc.dram_tensor("Xl", [C, HW], bf16, kind="ExternalInput").ap()
    Wqh_d = nc.dram_tensor("WqTh", [C, C8], bf16, kind="ExternalInput").ap()
    Wql_d = nc.dram_tensor("WqTl", [C, C8], bf16, kind="ExternalInput").ap()
    Wkh_d = nc.dram_tensor("WkTh", [C, C8], bf16, kind="ExternalInput").ap()
    Wkl_d = nc.dram_tensor("WkTl", [C, C8], bf16, kind="ExternalInput").ap()
    WvT_d = nc.dram_tensor("WvT", [C, C], bf16, kind="ExternalInput").ap()
    Wch_d = nc.dram_tensor("WcTh", [3, C8, C8], bf16, kind="ExternalInput").ap()
    Wcl_d = nc.dram_tensor("WcTl", [3, C8, C8], bf16, kind="ExternalInput").ap()
    bq_d = nc.dram_tensor("bq", [C8], f32, kind="ExternalInput").ap()
    bc_d = nc.dram_tensor("bc", [3, C8], f32, kind="ExternalInput").ap()
    bk_d = nc.dram_tensor("bkb", [C8], bf16, kind="ExternalInput").ap()
    bv_d = nc.dram_tensor("bvb", [C], bf16, kind="ExternalInput").ap()
    gam_d = nc.dram_tensor("gamma", [1], f32, kind="ExternalInput").ap()
    ones_d = nc.dram_tensor("ones", [P], bf16, kind="ExternalInput").ap()
    idf_d = nc.dram_tensor("identf", [P, P], f32, kind="ExternalInput").ap()
    idb_d = nc.dram_tensor("identb", [P, P], bf16, kind="ExternalInput").ap()
    comb_d = nc.dram_tensor("combined_out", [HW, C], f32, kind="ExternalOutput").ap()
    aout_d = nc.dram_tensor("attn_outs_out", [3, C, HW], f32, kind="ExternalOutput").ap()

    with tile.TileContext(nc) as tc:
        with tc.tile_pool(name="const", bufs=1) as const:
            identf = const.tile([P, P], f32)
            nc.sync.dma_start(out=identf, in_=idf_d)
            identb = const.tile([P, P], bf16)
            nc.sync.dma_start(out=identb, in_=idb_d)
            ones = const.tile([1, P], bf16)
            nc.sync.dma_start(out=ones, in_=ones_d.rearrange("(p n) -> p n", p=1))

            bq_sb = const.tile([P, 5], f32)
            nc.sync.dma_start(out=bq_sb[:, 0:4],
                              in_=bq_d[0:512].rearrange("(t p) -> p t", p=P))
            nc.sync.dma_start(out=bq_sb[0:64, 4:5],
                              in_=bq_d[512:576].rearrange("(p t) -> p t", t=1))
            bc_sb = const.tile([P, 3, 5], f32)
            for i in range(3):
                nc.sync.dma_start(out=bc_sb[:, i, 0:4],
                                  in_=bc_d[i, 0:512].rearrange("(t p) -> p t", p=P))
                nc.sync.dma_start(out=bc_sb[0:64, i, 4:5],
                                  in_=bc_d[i, 512:576].rearrange("(p t) -> p t", t=1))
            gam_sb = const.tile([P, 1], f32)
            nc.sync.dma_start(out=gam_sb,
                              in_=bass.AP(tensor=gam_d.tensor, offset=gam_d.offset,
                                          ap=[[0, P], [1, 1]]))

            qTh = const.tile([P, 5, HW], bf16)
            qTl = const.tile([P, 5, HW], bf16)
            kh_sb = const.tile([P, 5, C8], bf16)
            kl_sb = const.tile([P, 5, C8], bf16)
            attnT_sb = const.tile([P, 3, 5, HW], bf16)
            v_sb = const.tile([P, 5, C], bf16)

            with tc.tile_pool(name="xpool", bufs=1) as xp:
                Xh_sb = xp.tile([P, CT, HW], bf16)
                nc.sync.dma_start(out=Xh_sb, in_=Xh_d.rearrange("(ct p) n -> p ct n", p=P))
                Xl_sb = xp.tile([P, CT, HW], bf16)
                nc.sync.dma_start(out=Xl_sb, in_=Xl_d.rearrange("(ct p) n -> p ct n", p=P))

                # ---- Phase A1: qT[k, n] = Wq @ X (split-bf16) ----
                with tc.tile_pool(name="wq", bufs=2) as wqp, \
                     tc.tile_pool(name="qb", bufs=3) as qbp, \
                     tc.tile_pool(name="psA", bufs=4, space="PSUM") as psA:
                    for kt, (koff, ksz) in enumerate(NT):
                        wblks = []
                        for half in range(2):
                            wh = wqp.tile([P, 18, P], bf16, tag=f"wqh{half}")
                            wl = wqp.tile([P, 18, P], bf16, tag=f"wql{half}")
                            nc.sync.dma_start(
                                out=wh[:, :, 0:ksz],
                                in_=Wqh_d[half * 2304:(half + 1) * 2304, koff:koff + ksz]
                                    .rearrange("(ct p) k -> p ct k", p=P))
                            nc.sync.dma_start(
                                out=wl[:, :, 0:ksz],
                                in_=Wql_d[half * 2304:(half + 1) * 2304, koff:koff + ksz]
                                    .rearrange("(ct p) k -> p ct k", p=P))
                            wblks.append((wh, wl))
                        for (noff, nsz) in NCH:
                            ps = psA.tile([P, 288], f32, tag="qps")
                            for half in range(2):
                                wh, wl = wblks[half]
                                for ct in range(18):
                                    g = half * 18 + ct
                                    for j, (lhsT, rhs) in enumerate((
                                            (wh[:, ct, 0:ksz], Xh_sb[:, g, noff:noff + nsz]),
                                            (wh[:, ct, 0:ksz], Xl_sb[:, g, noff:noff + nsz]),
                                            (wl[:, ct, 0:ksz], Xh_sb[:, g, noff:noff + nsz]))):
                                        nc.tensor.matmul(ps[0:ksz, :], lhsT=lhsT, rhs=rhs,
                                                         start=(half == 0 and ct == 0 and j == 0),
                                                         stop=(half == 1 and ct == 17 and j == 2))
                            q32 = qbp.tile([P, 288], f32, tag="q32")
                            nc.scalar.activation(q32[0:ksz, :], ps[0:ksz, :], Act.Identity,
                                                 bias=bq_sb[0:ksz, kt:kt + 1])
                            nc.vector.tensor_copy(qTh[0:ksz, kt, noff:noff + nsz], q32[0:ksz, :])
                            nc.vector.tensor_sub(qTl[0:ksz, kt, noff:noff + nsz],
                                                 q32[0:ksz, :], qTh[0:ksz, kt, noff:noff + nsz])

                # ---- Phase A2: k[m, c] token-major (split-bf16) ----
                with tc.tile_pool(name="wk", bufs=2) as wkp, \
                     tc.tile_pool(name="kb", bufs=3) as kbp, \
                     tc.tile_pool(name="bkp", bufs=1) as bkp, \
                     tc.tile_pool(name="psA2", bufs=7, space="PSUM") as psA2:
                    bk_sb = bkp.tile([1, C8], bf16)
                    nc.sync.dma_start(out=bk_sb, in_=bk_d.rearrange("(p n) -> p n", p=1))
                    for (coff, csz) in NCH:
                        pss = [psA2.tile([P, 288], f32, tag="kps", name=f"kps{coff}_{j}")
                               for j in range(5)]
                        for mt, (moff, msz) in enumerate(NT):
                            nc.tensor.matmul(pss[mt][0:msz, :], lhsT=ones[0:1, 0:msz],
                                             rhs=bk_sb[:, coff:coff + csz],
                                             start=True, stop=False)
                        for quart in range(4):
                            wh = wkp.tile([P, 9, 288], bf16, tag="wkh")
                            wl = wkp.tile([P, 9, 288], bf16, tag="wkl")
                            nc.sync.dma_start(
                                out=wh[:, :, 0:csz],
                                in_=Wkh_d[quart * 1152:(quart + 1) * 1152, coff:coff + csz]
                                    .rearrange("(ct p) k -> p ct k", p=P))
                            nc.sync.dma_start(
                                out=wl[:, :, 0:csz],
                                in_=Wkl_d[quart * 1152:(quart + 1) * 1152, coff:coff + csz]
                                    .rearrange("(ct p) k -> p ct k", p=P))
                            for mt, (moff, msz) in enumerate(NT):
                                for ct in range(9):
                                    g = quart * 9 + ct
                                    last = (quart == 3 and ct == 8)
                                    nc.tensor.matmul(pss[mt][0:msz, :],
                                                     lhsT=Xh_sb[:, g, moff:moff + msz],
                                                     rhs=wh[:, ct, 0:csz],
                                                     start=False, stop=False)
                                    nc.tensor.matmul(pss[mt][0:msz, :],
                                                     lhsT=Xl_sb[:, g, moff:moff + msz],
                                                     rhs=wh[:, ct, 0:csz],
                                                     start=False, stop=False)
                                    nc.tensor.matmul(pss[mt][0:msz, :],
                                                     lhsT=Xh_sb[:, g, moff:moff + msz],
                                                     rhs=wl[:, ct, 0:csz],
                                                     start=False, stop=last)
                        for mt, (moff, msz) in enumerate(NT):
                            k32 = kbp.tile([P, 288], f32, tag="k32")
                            nc.scalar.activation(k32[0:msz, :], pss[mt][0:msz, :], Act.Copy)
                            nc.vector.tensor_copy(kh_sb[0:msz, mt, coff:coff + csz], k32[0:msz, :])
                            nc.vector.tensor_sub(kl_sb[0:msz, mt, coff:coff + csz],
                                                 k32[0:msz, :], kh_sb[0:msz, mt, coff:coff + csz])

                # ---- Phase B: v[t, d] token-major bf16, resident ----
                with tc.tile_pool(name="wv", bufs=2) as wvp, \
                     tc.tile_pool(name="bvp", bufs=1) as bvp, \
                     tc.tile_pool(name="psB", bufs=7, space="PSUM") as psB:
                    bv_sb = bvp.tile([1, C], bf16)
                    nc.sync.dma_start(out=bv_sb, in_=bv_d.rearrange("(p n) -> p n", p=1))
                    for dch in range(9):
                        doff = dch * 512
                        pss = [psB.tile([P, 512], f32, tag="vps", name=f"vps{dch}_{j}")
                               for j in range(5)]
                        for mt, (moff, msz) in enumerate(NT):
                            nc.tensor.matmul(pss[mt][0:msz, :], lhsT=ones[0:1, 0:msz],
                                             rhs=bv_sb[:, doff:doff + 512],
                                             start=True, stop=False)
                        for quart in range(4):
                            wv_blk = wvp.tile([P, 9, 512], bf16, tag="wv")
                            nc.sync.dma_start(
                                out=wv_blk,
                                in_=WvT_d[quart * 1152:(quart + 1) * 1152, doff:doff + 512]
                                    .rearrange("(ct p) d -> p ct d", p=P))
                            for mt, (moff, msz) in enumerate(NT):
                                for ct in range(9):
                                    nc.tensor.matmul(
                                        pss[mt][0:msz, :],
                                        lhsT=Xh_sb[:, quart * 9 + ct, moff:moff + msz],
                                        rhs=wv_blk[:, ct, :],
                                        start=False, stop=(quart == 3 and ct == 8))
                        for mt, (moff, msz) in enumerate(NT):
                            nc.vector.tensor_copy(v_sb[0:msz, mt, doff:doff + 512],
                                                  pss[mt][0:msz, :])

            # ---- Phase C: components -> attnT (split-bf16 cq & energy) ----
            with tc.tile_pool(name="wc", bufs=2) as wcp, \
                 tc.tile_pool(name="cq", bufs=2) as cqp, \
                 tc.tile_pool(name="cqb", bufs=3) as cqbp, \
                 tc.tile_pool(name="at", bufs=3) as atp, \
                 tc.tile_pool(name="sm", bufs=8) as smp, \
                 tc.tile_pool(name="psC", bufs=2, space="PSUM") as psC, \
                 tc.tile_pool(name="psE", bufs=4, space="PSUM") as psE, \
                 tc.tile_pool(name="psT", bufs=2, space="PSUM") as psT:
                for i in range(3):
                    wch = wcp.tile([P, 5, C8], bf16, tag="wch")
                    wcl = wcp.tile([P, 5, C8], bf16, tag="wcl")
                    nc.sync.dma_start(out=wch[:, 0:4, :],
                                      in_=Wch_d[i, 0:512, :].rearrange("(t p) m -> p t m", p=P))
                    nc.sync.dma_start(out=wch[0:64, 4, :], in_=Wch_d[i, 512:576, :])
                    nc.sync.dma_start(out=wcl[:, 0:4, :],
                                      in_=Wcl_d[i, 0:512, :].rearrange("(t p) m -> p t m", p=P))
                    nc.sync.dma_start(out=wcl[0:64, 4, :], in_=Wcl_d[i, 512:576, :])
                    cqh = cqp.tile([P, 5, HW], bf16, tag="cqh")
                    cql = cqp.tile([P, 5, HW], bf16, tag="cql")
                    for mt, (moff, msz) in enumerate(NT):
                        for (noff, nsz) in NCH:
                            ps = psC.tile([P, 288], f32, tag="cqps")
                            for kt, (koff, ksz) in enumerate(NT):
                                for j, (lhsT, rhs) in enumerate((
                                        (wch[0:ksz, kt, moff:moff + msz], qTh[0:ksz, kt, noff:noff + nsz]),
                                        (wch[0:ksz, kt, moff:moff + msz], qTl[0:ksz, kt, noff:noff + nsz]),
                                        (wcl[0:ksz, kt, moff:moff + msz], qTh[0:ksz, kt, noff:noff + nsz]))):
                                    nc.tensor.matmul(ps[0:msz, :], lhsT=lhsT, rhs=rhs,
                                                     start=(kt == 0 and j == 0),
                                                     stop=(kt == 4 and j == 2))
                            cq32 = cqbp.tile([P, 288], f32, tag="cq32")
                            nc.scalar.activation(cq32[0:msz, :], ps[0:msz, :], Act.Identity,
                                                 bias=bc_sb[0:msz, i, mt:mt + 1])
                            nc.vector.tensor_copy(cqh[0:msz, mt, noff:noff + nsz], cq32[0:msz, :])
                            nc.vector.tensor_sub(cql[0:msz, mt, noff:noff + nsz],
                                                 cq32[0:msz, :], cqh[0:msz, mt, noff:noff + nsz])
                    for nt, (noff, nsz) in enumerate(NT):
                        eps = []
                        for (coff, csz) in NCH:
                            ps = psE.tile([P, 288], f32, tag="eps",
                                          name=f"eps{i}_{nt}_{coff}")
                            for mt, (moff, msz) in enumerate(NT):
                                for j, (lhsT, rhs) in enumerate((
                                        (cqh[0:msz, mt, noff:noff + nsz], kh_sb[0:msz, mt, coff:coff + csz]),
                                        (cqh[0:msz, mt, noff:noff + nsz], kl_sb[0:msz, mt, coff:coff + csz]),
                                        (cql[0:msz, mt, noff:noff + nsz], kh_sb[0:msz, mt, coff:coff + csz]))):
                                    nc.tensor.matmul(ps[0:nsz, :], lhsT=lhsT, rhs=rhs,
                                                     start=(mt == 0 and j == 0),
                                                     stop=(mt == 4 and j == 2))
                            eps.append(ps)
                        nm0 = smp.tile([P, 1], f32, tag="nm0")
                        nm1 = smp.tile([P, 1], f32, tag="nm1")
                        nc.vector.tensor_reduce(nm0[0:nsz, :], eps[0][0:nsz, :],
                                                axis=AxisX, op=Alu.max, negate=True)
                        nc.vector.tensor_reduce(nm1[0:nsz, :], eps[1][0:nsz, :],
                                                axis=AxisX, op=Alu.max, negate=True)
                        nm = smp.tile([P, 1], f32, tag="nm")
                        nc.vector.tensor_tensor(nm[0:nsz, :], nm0[0:nsz, :],
                                                nm1[0:nsz, :], op=Alu.min)
                        att = atp.tile([P, HW], bf16, tag="att")
                        s0 = smp.tile([P, 1], f32, tag="s0")
                        s1 = smp.tile([P, 1], f32, tag="s1")
                        nc.scalar.activation(att[0:nsz, 0:288], eps[0][0:nsz, :],
                                             Act.Exp, bias=nm[0:nsz, :],
                                             accum_out=s0[0:nsz, :])
                        nc.scalar.activation(att[0:nsz, 288:576], eps[1][0:nsz, :],
                                             Act.Exp, bias=nm[0:nsz, :],
                                             accum_out=s1[0:nsz, :])
                        ssum = smp.tile([P, 1], f32, tag="ss")
                        rec = smp.tile([P, 1], f32, tag="rec")
                        nc.vector.tensor_add(ssum[0:nsz, :], s0[0:nsz, :], s1[0:nsz, :])
                        nc.vector.reciprocal(rec[0:nsz, :], ssum[0:nsz, :])
                        nc.vector.tensor_scalar_mul(att[0:nsz, :], att[0:nsz, :],
                                                    rec[0:nsz, 0:1])
                        for ctb, (coff, csz) in enumerate(NT):
                            pt = psT.tile([P, P], bf16, tag="pt")
                            nc.tensor.transpose(pt[0:csz, 0:nsz],
                                                att[0:nsz, coff:coff + csz],
                                                identb[0:nsz, 0:nsz])
                            nc.vector.tensor_copy(
                                attnT_sb[0:csz, i, ctb, noff:noff + nsz],
                                pt[0:csz, 0:nsz])

            # ---- Phase D: out_i^T, combined ----
            with tc.tile_pool(name="xs", bufs=3) as xsp, \
                 tc.tile_pool(name="os", bufs=2) as osp, \
                 tc.tile_pool(name="stg", bufs=2) as stgp, \
                 tc.tile_pool(name="psD", bufs=4, space="PSUM") as psD, \
                 tc.tile_pool(name="psDT", bufs=2, space="PSUM") as psDT:
                stage = {}
                for dt in range(CT):
                    doff = dt * P
                    outs = []
                    for i in range(3):
                        ot = osp.tile([P, HW], f32, tag=f"os{i}")
                        for (noff, nsz) in NCH:
                            ps = psD.tile([P, 288], f32, tag="dps")
                            for tt, (toff, tsz) in enumerate(NT):
                                nc.tensor.matmul(
                                    ps[:, :], lhsT=v_sb[0:tsz, tt, doff:doff + P],
                                    rhs=attnT_sb[0:tsz, i, tt, noff:noff + nsz],
                                    start=(tt == 0), stop=(tt == 4))
                            nc.scalar.activation(ot[:, noff:noff + nsz], ps[:, :],
                                                 Act.Copy)
                        nc.sync.dma_start(out=aout_d[i, doff:doff + P, :], in_=ot)
                        outs.append(ot)
                    xs = xsp.tile([P, HW], f32, tag="xs")
                    nc.sync.dma_start(out=xs, in_=X32_d[doff:doff + P, :])
                    u = osp.tile([P, HW], f32, tag="u")
                    nc.gpsimd.tensor_add(u, outs[0], outs[1])
                    u2 = osp.tile([P, HW], f32, tag="u2")
                    nc.gpsimd.tensor_add(u2, u, outs[2])
                    cmb = osp.tile([P, HW], f32, tag="cmb")
                    nc.vector.scalar_tensor_tensor(
                        cmb, u2, gam_sb[:, 0:1], xs, op0=Alu.mult, op1=Alu.add)
                    grp = dt % 4
                    if grp == 0:
                        stage = {nt: stgp.tile([P, 512], f32, tag=f"st{nt}",
                                               name=f"st{nt}_{dt}")
                                 for nt in range(5)}
                    for nt, (noff, nsz) in enumerate(NT):
                        pt = psDT.tile([P, P], f32, tag="pdt")
                        nc.tensor.transpose(pt[0:nsz, 0:P], cmb[:, noff:noff + nsz],
                                            identf[:, :])
                        nc.vector.tensor_copy(stage[nt][0:nsz, grp * P:(grp + 1) * P],
                                              pt[0:nsz, 0:P])
                    if grp == 3:
                        for nt, (noff, nsz) in enumerate(NT):
                            nc.sync.dma_start(
                                out=comb_d[noff:noff + nsz, (dt - 3) * P:(dt + 1) * P],
                                in_=stage[nt][0:nsz, :])

    nc.compile()
    return nc


def run_spmd(nc, in_maps, time_iters=0):
    """Execute on the 8 axon cores via PJRT (no donation so the compiled fn can
    be re-invoked on device-resident buffers for timing)."""
    import jax
    import concourse.mybir as mybir
    from concourse import bass2jax
    from jax.sharding import Mesh, NamedSharding, PartitionSpec
    from jax.experimental.shard_map import shard_map

    bass2jax.install_neuronx_cc_hook()
    partition_name = nc.partition_id_tensor.name if nc.partition_id_tensor else None
    in_names, out_names, out_avals = [], [], []
    for alloc in nc.m.functions[0].allocations:
        if not isinstance(alloc, mybir.MemoryLocationSet):
            continue
        name = alloc.memorylocations[0].name
        if alloc.kind == "ExternalInput":
            if name != partition_name:
                in_names.append(name)
        elif alloc.kind == "ExternalOutput":
            out_names.append(name)
            out_avals.append(jax.core.ShapedArray(tuple(alloc.tensor_shape),
                                                  mybir.dt.np(alloc.dtype)))
    n_params = len(in_names)
    all_in = in_names + out_names
    if partition_name is not None:
        all_in = all_in + [partition_name]

    def _body(*args):
        operands = list(args)
        if partition_name is not None:
            operands.append(bass2jax.partition_id_tensor())
        return tuple(bass2jax._bass_exec_p.bind(
            *operands, out_avals=tuple(out_avals), in_names=tuple(all_in),
            out_names=tuple(out_names), lowering_input_output_aliases=(),
            sim_require_finite=True, sim_require_nnan=True, nc=nc))

    devices = jax.devices()[:N_CORES]
    mesh = Mesh(np.asarray(devices), ("core",))
    nspec = (PartitionSpec("core"),)
    sharded = jax.jit(shard_map(_body, mesh=mesh,
                                in_specs=nspec * (n_params + len(out_names)),
                                out_specs=nspec * len(out_names), check_rep=False),
                      keep_unused=True)
    sh = NamedSharding(mesh, PartitionSpec("core"))
    dev_in = [jax.device_put(
        np.concatenate([np.asarray(in_maps[c][nm]) for c in range(N_CORES)], axis=0),
        sh) for nm in in_names]
    dev_zero = [jax.device_put(
        np.zeros((N_CORES * a.shape[0], *a.shape[1:]), a.dtype), sh)
        for a in out_avals]
    out_arrs = sharded(*dev_in, *dev_zero)
    jax.block_until_ready(out_arrs)
    per_iter_ns = None
    if time_iters:
        import time
        t0 = time.perf_counter()
        for _ in range(time_iters):
            out_arrs = sharded(*dev_in, *dev_zero)
        jax.block_until_ready(out_arrs)
        per_iter_ns = (time.perf_counter() - t0) / time_iters * 1e9
    results = [{name: np.asarray(out_arrs[i]).reshape(N_CORES, *out_avals[i].shape)[c]
                for i, name in enumerate(out_names)} for c in range(N_CORES)]
    return results, per_iter_ns


def make_in_maps(x, Wq, bq, Wk, bk, Wv, bv, Wc, bc, gamma):
    import ml_dtypes
    bfdt = ml_dtypes.bfloat16
    f = lambda a: np.ascontiguousarray(np.asarray(a, dtype=np.float32))

    def split(a):
        a = f(a)
        hi = a.astype(bfdt)
        lo = (a - hi.astype(np.float32)).astype(bfdt)
        return np.ascontiguousarray(hi), np.ascontiguousarray(lo)

    WqTh, WqTl = split(np.asarray(Wq, np.float32).T)
    WkTh, WkTl = split(np.asarray(Wk, np.float32).T)
    WvT = np.ascontiguousarray(f(np.asarray(Wv, np.float32).T).astype(bfdt))
    WcTh, WcTl = split(np.transpose(np.asarray(Wc, np.float32), (0, 2, 1)))
    shared = {"WqTh": WqTh, "WqTl": WqTl, "WkTh": WkTh, "WkTl": WkTl,
              "WvT": WvT, "WcTh": WcTh, "WcTl": WcTl,
              "bq": f(bq), "bc": f(bc),
              "bkb": f(bk).astype(bfdt), "bvb": f(bv).astype(bfdt),
              "gamma": f(gamma), "ones": np.ones(P, dtype=bfdt),
              "identf": np.eye(P, dtype=np.float32),
              "identb": np.eye(P, dtype=bfdt)}
    in_maps = []
    for b in range(B):
        X32 = f(np.asarray(x[b], np.float32).reshape(HW, C).T)
        Xh, Xl = split(X32)
        in_maps.append({"X32": X32, "Xh": Xh, "Xl": Xl, **shared})
    return in_maps


def assemble(results):
    combined = np.stack([results[b]["combined_out"].reshape(H, Wd, C)
                         for b in range(B)], axis=0)
    attn_outs = np.stack([results[b]["attn_outs_out"].reshape(3, C, H, Wd)
                          for b in range(B)], axis=1)
    return combined, attn_outs


def kernel(x, Wq, bq, Wk, bk, Wv, bv, Wc, bc, gamma):
    nc = build_nc()
    in_maps = make_in_maps(x, Wq, bq, Wk, bk, Wv, bv, Wc, bc, gamma)
    results, _ = run_spmd(nc, in_maps)
    return assemble(results)


# revision 14
# speedup vs baseline: 13.9884x; 5.3303x over previous
"""Trainium2 Bass kernel for ComponentBasedHierarchicalAttention.

Strategy: data-parallel over batch (B=8, one batch element per NeuronCore).
All matmuls in bf16 (measured ~6x faster than fp32r on this silicon). The
softmax-critical chain (q, k, cq, energy) uses split-bf16: operands are
represented as hi+lo bf16 pairs and each matmul computes the 3-term expansion
hi@hi + hi@lo + lo@hi with fp32 PSUM accumulation (~2^-17 operand precision),
because the energy logits have std ~21 and softmax amplifies absolute logit
error. The v projection and attn@v run in plain bf16 (linear error paths).

Per core, X = x[b]^T [C=4608, HW=576] channel-major:
  A1: qT[k,n] = Wq @ X          (channel-major q, split-bf16, +bq via ACT bias)
  A2: k[m,c]  = (X^T Wk^T)      (token-major k, split-bf16, +bk via ones-row MM)
  B:  v[t,d]  = (X^T Wv^T)      (token-major v, bf16, resident in SBUF)
  C:  per component i: cqT = Wc_i @ q^T; energy = cq_i @ k (token-major);
      softmax along free dim; attn -> attnT via PE transpose
  D:  per 128-channel block: out_i^T[d,n] (channel-major -> attn_outs);
      combined^T = gamma*sum_i out_i^T + X; PE transpose -> combined

build_nc(reps=N) repeats the whole pipeline N times inside one NEFF — used by
test.py to measure true device time as (T(N) - T(1)) / (N - 1), independent of
the ~2.5 ms PJRT/axon per-call dispatch floor.
"""

import sys

if "/opt/trn_rl_repo" not in sys.path:
    sys.path.insert(0, "/opt/trn_rl_repo")

import numpy as np

B, H, Wd, C = 8, 24, 24, 4608
HW = H * Wd          # 576
C8 = 576
P = 128
NT = [(0, 128), (128, 128), (256, 128), (384, 128), (512, 64)]   # 576 partition tiles
CT = C // P          # 36 channel tiles
NCH = [(0, 288), (288, 288)]                                     # 576 free-dim chunks
N_CORES = 8


def build_nc(reps=1):
    import concourse.bass as bass
    import concourse.mybir as mybir
    import concourse.tile as tile
    from concourse import bacc

    f32 = mybir.dt.float32
    bf16 = mybir.dt.bfloat16
    Act = mybir.ActivationFunctionType
    Alu = mybir.AluOpType
    AxisX = mybir.AxisListType.X

    nc = bacc.Bacc("TRN2", target_bir_lowering=False, debug=False,
                   num_devices=N_CORES)

    X32_d = nc.dram_tensor("X32", [C, HW], f32, kind="ExternalInput").ap()
    Xh_d = nc.dram_tensor("Xh", [C, HW], bf16, kind="ExternalInput").ap()
    Xl_d = nc.dram_tensor("Xl", [C, HW], bf16, kind="ExternalInput").ap()
    Wqh_d = nc.dram_tensor("WqTh", [C, C8], bf16, kind="ExternalInput").ap()
    Wql_d = nc.dram_tensor("WqTl", [C, C8], bf16, kind="ExternalInput").ap()
    Wkh_d = nc.dram_tensor("WkTh", [C, C8], bf16, kind="ExternalInput").ap()
    Wkl_d = nc.dram_tensor("WkTl", [C, C8], bf16, kind="ExternalInput").ap()
    WvT_d = nc.dram_tensor("WvT", [C, C], bf16, kind="ExternalInput").ap()
    Wch_d = nc.dram_tensor("WcTh", [3, C8, C8], bf16, kind="ExternalInput").ap()
    Wcl_d = nc.dram_tensor("WcTl", [3, C8, C8], bf16, kind="ExternalInput").ap()
    bq_d = nc.dram_tensor("bq", [C8], f32, kind="ExternalInput").ap()
    bc_d = nc.dram_tensor("bc", [3, C8], f32, kind="ExternalInput").ap()
    bk_d = nc.dram_tensor("bkb", [C8], bf16, kind="ExternalInput").ap()
    bv_d = nc.dram_tensor("bvb", [C], bf16, kind="ExternalInput").ap()
    gam_d = nc.dram_tensor("gamma", [1], f32, kind="ExternalInput").ap()
    ones_d = nc.dram_tensor("ones", [P], bf16, kind="ExternalInput").ap()
    idf_d = nc.dram_tensor("identf", [P, P], f32, kind="ExternalInput").ap()
    idb_d = nc.dram_tensor("identb", [P, P], bf16, kind="ExternalInput").ap()
    comb_d = nc.dram_tensor("combined_out", [HW, C], f32, kind="ExternalOutput").ap()
    aout_d = nc.dram_tensor("attn_outs_out", [3, C, HW], f32, kind="ExternalOutput").ap()

    with tile.TileContext(nc) as tc:
        with tc.tile_pool(name="const", bufs=1) as const:
            identf = const.tile([P, P], f32)
            nc.sync.dma_start(out=identf, in_=idf_d)
            identb = const.tile([P, P], bf16)
            nc.sync.dma_start(out=identb, in_=idb_d)
            ones = const.tile([1, P], bf16)
            nc.sync.dma_start(out=ones, in_=ones_d.rearrange("(p n) -> p n", p=1))

            bq_sb = const.tile([P, 5], f32)
            nc.sync.dma_start(out=bq_sb[:, 0:4],
                              in_=bq_d[0:512].rearrange("(t p) -> p t", p=P))
            nc.sync.dma_start(out=bq_sb[0:64, 4:5],
                              in_=bq_d[512:576].rearrange("(p t) -> p t", t=1))
            bc_sb = const.tile([P, 3, 5], f32)
            for i in range(3):
                nc.sync.dma_start(out=bc_sb[:, i, 0:4],
                                  in_=bc_d[i, 0:512].rearrange("(t p) -> p t", p=P))
                nc.sync.dma_start(out=bc_sb[0:64, i, 4:5],
                                  in_=bc_d[i, 512:576].rearrange("(p t) -> p t", t=1))
            gam_sb = const.tile([P, 1], f32)
            nc.sync.dma_start(out=gam_sb,
                              in_=bass.AP(tensor=gam_d.tensor, offset=gam_d.offset,
                                          ap=[[0, P], [1, 1]]))

            qTh = const.tile([P, 5, HW], bf16)
            qTl = const.tile([P, 5, HW], bf16)
            kh_sb = const.tile([P, 5, C8], bf16)
            kl_sb = const.tile([P, 5, C8], bf16)
            attnT_sb = const.tile([P, 3, 5, HW], bf16)
            v_sb = const.tile([P, 5, C], bf16)

            for rep in range(reps):
                _one_pass(nc, tc, rep, locals())

    nc.compile()
    return nc


def _one_pass(nc, tc, rep, env):
    import concourse.bass as bass
    import concourse.mybir as mybir

    f32 = mybir.dt.float32
    bf16 = mybir.dt.bfloat16
    Act = mybir.ActivationFunctionType
    Alu = mybir.AluOpType
    AxisX = mybir.AxisListType.X

    Xh_d, Xl_d, X32_d = env["Xh_d"], env["Xl_d"], env["X32_d"]
    Wqh_d, Wql_d = env["Wqh_d"], env["Wql_d"]
    Wkh_d, Wkl_d = env["Wkh_d"], env["Wkl_d"]
    WvT_d, Wch_d, Wcl_d = env["WvT_d"], env["Wch_d"], env["Wcl_d"]
    bk_d, bv_d = env["bk_d"], env["bv_d"]
    comb_d, aout_d = env["comb_d"], env["aout_d"]
    identf, identb, ones = env["identf"], env["identb"], env["ones"]
    bq_sb, bc_sb, gam_sb = env["bq_sb"], env["bc_sb"], env["gam_sb"]
    qTh, qTl = env["qTh"], env["qTl"]
    kh_sb, kl_sb = env["kh_sb"], env["kl_sb"]
    attnT_sb, v_sb = env["attnT_sb"], env["v_sb"]

    with tc.tile_pool(name=f"xpool{rep}", bufs=1) as xp:
        Xh_sb = xp.tile([P, CT, HW], bf16, name=f"Xh_sb{rep}")
        nc.sync.dma_start(out=Xh_sb, in_=Xh_d.rearrange("(ct p) n -> p ct n", p=P))
        Xl_sb = xp.tile([P, CT, HW], bf16, name=f"Xl_sb{rep}")
        nc.sync.dma_start(out=Xl_sb, in_=Xl_d.rearrange("(ct p) n -> p ct n", p=P))

        # ---- Phase A1: qT[k, n] = Wq @ X (split-bf16) ----
        with tc.tile_pool(name=f"wq{rep}", bufs=2) as wqp, \
             tc.tile_pool(name=f"qb{rep}", bufs=3) as qbp, \
             tc.tile_pool(name=f"psA{rep}", bufs=4, space="PSUM") as psA:
            for kt, (koff, ksz) in enumerate(NT):
                wblks = []
                for half in range(2):
                    wh = wqp.tile([P, 18, P], bf16, tag=f"wqh{half}", name=f"wqh{rep}_{kt}_{half}")
                    wl = wqp.tile([P, 18, P], bf16, tag=f"wql{half}", name=f"wql{rep}_{kt}_{half}")
                    nc.sync.dma_start(
                        out=wh[:, :, 0:ksz],
                        in_=Wqh_d[half * 2304:(half + 1) * 2304, koff:koff + ksz]
                            .rearrange("(ct p) k -> p ct k", p=P))
                    nc.sync.dma_start(
                        out=wl[:, :, 0:ksz],
                        in_=Wql_d[half * 2304:(half + 1) * 2304, koff:koff + ksz]
                            .rearrange("(ct p) k -> p ct k", p=P))
                    wblks.append((wh, wl))
                for (noff, nsz) in NCH:
                    ps = psA.tile([P, 288], f32, tag="qps", name=f"qps{rep}_{kt}_{noff}")
                    for half in range(2):
                        wh, wl = wblks[half]
                        for ct in range(18):
                            g = half * 18 + ct
                            for j, (lhsT, rhs) in enumerate((
                                    (wh[:, ct, 0:ksz], Xh_sb[:, g, noff:noff + nsz]),
                                    (wh[:, ct, 0:ksz], Xl_sb[:, g, noff:noff + nsz]),
                                    (wl[:, ct, 0:ksz], Xh_sb[:, g, noff:noff + nsz]))):
                                nc.tensor.matmul(ps[0:ksz, :], lhsT=lhsT, rhs=rhs,
                                                 start=(half == 0 and ct == 0 and j == 0),
                                                 stop=(half == 1 and ct == 17 and j == 2))
                    q32 = qbp.tile([P, 288], f32, tag="q32", name=f"q32_{rep}_{kt}_{noff}")
                    nc.scalar.activation(q32[0:ksz, :], ps[0:ksz, :], Act.Identity,
                                         bias=bq_sb[0:ksz, kt:kt + 1])
                    nc.vector.tensor_copy(qTh[0:ksz, kt, noff:noff + nsz], q32[0:ksz, :])
                    nc.vector.tensor_sub(qTl[0:ksz, kt, noff:noff + nsz],
                                         q32[0:ksz, :], qTh[0:ksz, kt, noff:noff + nsz])

        # ---- Phase A2: k[m, c] token-major (split-bf16) ----
        with tc.tile_pool(name=f"wk{rep}", bufs=2) as wkp, \
             tc.tile_pool(name=f"kb{rep}", bufs=3) as kbp, \
             tc.tile_pool(name=f"bkp{rep}", bufs=1) as bkp, \
             tc.tile_pool(name=f"psA2{rep}", bufs=7, space="PSUM") as psA2:
            bk_sb = bkp.tile([1, C8], bf16, name=f"bk_sb{rep}")
            nc.sync.dma_start(out=bk_sb, in_=bk_d.rearrange("(p n) -> p n", p=1))
            for (coff, csz) in NCH:
                pss = [psA2.tile([P, 288], f32, tag="kps", name=f"kps{rep}_{coff}_{j}")
                       for j in range(5)]
                for mt, (moff, msz) in enumerate(NT):
                    nc.tensor.matmul(pss[mt][0:msz, :], lhsT=ones[0:1, 0:msz],
                                     rhs=bk_sb[:, coff:coff + csz],
                                     start=True, stop=False)
                for quart in range(4):
                    wh = wkp.tile([P, 9, 288], bf16, tag="wkh", name=f"wkh{rep}_{coff}_{quart}")
                    wl = wkp.tile([P, 9, 288], bf16, tag="wkl", name=f"wkl{rep}_{coff}_{quart}")
                    nc.sync.dma_start(
                        out=wh[:, :, 0:csz],
                        in_=Wkh_d[quart * 1152:(quart + 1) * 1152, coff:coff + csz]
                            .rearrange("(ct p) k -> p ct k", p=P))
                    nc.sync.dma_start(
                        out=wl[:, :, 0:csz],
                        in_=Wkl_d[quart * 1152:(quart + 1) * 1152, coff:coff + csz]
                            .rearrange("(ct p) k -> p ct k", p=P))
                    for mt, (moff, msz) in enumerate(NT):
                        for ct in range(9):
                            g = quart * 9 + ct
                            last = (quart == 3 and ct == 8)
                            nc.tensor.matmul(pss[mt][0:msz, :],
                                             lhsT=Xh_sb[:, g, moff:moff + msz],
                                             rhs=wh[:, ct, 0:csz],
                                             start=False, stop=False)
                            nc.tensor.matmul(pss[mt][0:msz, :],
                                             lhsT=Xl_sb[:, g, moff:moff + msz],
                                             rhs=wh[:, ct, 0:csz],
                                             start=False, stop=False)
                            nc.tensor.matmul(pss[mt][0:msz, :],
                                             lhsT=Xh_sb[:, g, moff:moff + msz],
                                             rhs=wl[:, ct, 0:csz],
                                             start=False, stop=last)
                for mt, (moff, msz) in enumerate(NT):
                    k32 = kbp.tile([P, 288], f32, tag="k32", name=f"k32_{rep}_{coff}_{mt}")
                    nc.scalar.activation(k32[0:msz, :], pss[mt][0:msz, :], Act.Copy)
                    nc.vector.tensor_copy(kh_sb[0:msz, mt, coff:coff + csz], k32[0:msz, :])
                    nc.vector.tensor_sub(kl_sb[0:msz, mt, coff:coff + csz],
                                         k32[0:msz, :], kh_sb[0:msz, mt, coff:coff + csz])

        # ---- Phase B: v[t, d] token-major bf16, resident ----
        with tc.tile_pool(name=f"wv{rep}", bufs=2) as wvp, \
             tc.tile_pool(name=f"bvp{rep}", bufs=1) as bvp, \
             tc.tile_pool(name=f"psB{rep}", bufs=7, space="PSUM") as psB:
            bv_sb = bvp.tile([1, C], bf16, name=f"bv_sb{rep}")
            nc.sync.dma_start(out=bv_sb, in_=bv_d.rearrange("(p n) -> p n", p=1))
            for dch in range(9):
                doff = dch * 512
                pss = [psB.tile([P, 512], f32, tag="vps", name=f"vps{rep}_{dch}_{j}")
                       for j in range(5)]
                for mt, (moff, msz) in enumerate(NT):
                    nc.tensor.matmul(pss[mt][0:msz, :], lhsT=ones[0:1, 0:msz],
                                     rhs=bv_sb[:, doff:doff + 512],
                                     start=True, stop=False)
                for quart in range(4):
                    wv_blk = wvp.tile([P, 9, 512], bf16, tag="wv", name=f"wv{rep}_{dch}_{quart}")
                    nc.sync.dma_start(
                        out=wv_blk,
                        in_=WvT_d[quart * 1152:(quart + 1) * 1152, doff:doff + 512]
                            .rearrange("(ct p) d -> p ct d", p=P))
                    for mt, (moff, msz) in enumerate(NT):
                        for ct in range(9):
                            nc.tensor.matmul(
                                pss[mt][0:msz, :],
                                lhsT=Xh_sb[:, quart * 9 + ct, moff:moff + msz],
                                rhs=wv_blk[:, ct, :],
                                start=False, stop=(quart == 3 and ct == 8))
                for mt, (moff, msz) in enumerate(NT):
                    nc.vector.tensor_copy(v_sb[0:msz, mt, doff:doff + 512],
                                          pss[mt][0:msz, :])

    # ---- Phase C: components -> attnT (split-bf16 cq & energy) ----
    with tc.tile_pool(name=f"wc{rep}", bufs=2) as wcp, \
         tc.tile_pool(name=f"cq{rep}", bufs=2) as cqp, \
         tc.tile_pool(name=f"cqb{rep}", bufs=3) as cqbp, \
         tc.tile_pool(name=f"at{rep}", bufs=3) as atp, \
         tc.tile_pool(name=f"sm{rep}", bufs=8) as smp, \
         tc.tile_pool(name=f"psC{rep}", bufs=2, space="PSUM") as psC, \
         tc.tile_pool(name=f"psE{rep}", bufs=4, space="PSUM") as psE, \
         tc.tile_pool(name=f"psT{rep}", bufs=2, space="PSUM") as psT:
        for i in range(3):
            wch = wcp.tile([P, 5, C8], bf16, tag="wch", name=f"wch{rep}_{i}")
            wcl = wcp.tile([P, 5, C8], bf16, tag="wcl", name=f"wcl{rep}_{i}")
            nc.sync.dma_start(out=wch[:, 0:4, :],
                              in_=Wch_d[i, 0:512, :].rearrange("(t p) m -> p t m", p=P))
            nc.sync.dma_start(out=wch[0:64, 4, :], in_=Wch_d[i, 512:576, :])
            nc.sync.dma_start(out=wcl[:, 0:4, :],
                              in_=Wcl_d[i, 0:512, :].rearrange("(t p) m -> p t m", p=P))
            nc.sync.dma_start(out=wcl[0:64, 4, :], in_=Wcl_d[i, 512:576, :])
            cqh = cqp.tile([P, 5, HW], bf16, tag="cqh", name=f"cqh{rep}_{i}")
            cql = cqp.tile([P, 5, HW], bf16, tag="cql", name=f"cql{rep}_{i}")
            for mt, (moff, msz) in enumerate(NT):
                for (noff, nsz) in NCH:
                    ps = psC.tile([P, 288], f32, tag="cqps", name=f"cqps{rep}_{i}_{mt}_{noff}")
                    for kt, (koff, ksz) in enumerate(NT):
                        for j, (lhsT, rhs) in enumerate((
                                (wch[0:ksz, kt, moff:moff + msz], qTh[0:ksz, kt, noff:noff + nsz]),
                                (wch[0:ksz, kt, moff:moff + msz], qTl[0:ksz, kt, noff:noff + nsz]),
                                (wcl[0:ksz, kt, moff:moff + msz], qTh[0:ksz, kt, noff:noff + nsz]))):
                            nc.tensor.matmul(ps[0:msz, :], lhsT=lhsT, rhs=rhs,
                                             start=(kt == 0 and j == 0),
                                             stop=(kt == 4 and j == 2))
                    cq32 = cqbp.tile([P, 288], f32, tag="cq32", name=f"cq32_{rep}_{i}_{mt}_{noff}")
                    nc.scalar.activation(cq32[0:msz, :], ps[0:msz, :], Act.Identity,
                                         bias=bc_sb[0:msz, i, mt:mt + 1])
                    nc.vector.tensor_copy(cqh[0:msz, mt, noff:noff + nsz], cq32[0:msz, :])
                    nc.vector.tensor_sub(cql[0:msz, mt, noff:noff + nsz],
                                         cq32[0:msz, :], cqh[0:msz, mt, noff:noff + nsz])
            for nt, (noff, nsz) in enumerate(NT):
                eps = []
                for (coff, csz) in NCH:
                    ps = psE.tile([P, 288], f32, tag="eps", name=f"eps{rep}_{i}_{nt}_{coff}")
                    for mt, (moff, msz) in enumerate(NT):
                        for j, (lhsT, rhs) in enumerate((
                                (cqh[0:msz, mt, noff:noff + nsz], kh_sb[0:msz, mt, coff:coff + csz]),
                                (cqh[0:msz, mt, noff:noff + nsz], kl_sb[0:msz, mt, coff:coff + csz]),
                                (cql[0:msz, mt, noff:noff + nsz], kh_sb[0:msz, mt, coff:coff + csz]))):
                            nc.tensor.matmul(ps[0:nsz, :], lhsT=lhsT, rhs=rhs,
                                             start=(mt == 0 and j == 0),
                                             stop=(mt == 4 and j == 2))
                    eps.append(ps)
                nm0 = smp.tile([P, 1], f32, tag="nm0", name=f"nm0_{rep}_{i}_{nt}")
                nm1 = smp.tile([P, 1], f32, tag="nm1", name=f"nm1_{rep}_{i}_{nt}")
                nc.vector.tensor_reduce(nm0[0:nsz, :], eps[0][0:nsz, :],
                                        axis=AxisX, op=Alu.max, negate=True)
                nc.vector.tensor_reduce(nm1[0:nsz, :], eps[1][0:nsz, :],
                                        axis=AxisX, op=Alu.max, negate=True)
                nm = smp.tile([P, 1], f32, tag="nm", name=f"nm_{rep}_{i}_{nt}")
                nc.vector.tensor_tensor(nm[0:nsz, :], nm0[0:nsz, :],
                                        nm1[0:nsz, :], op=Alu.min)
                att = atp.tile([P, HW], bf16, tag="att", name=f"att{rep}_{i}_{nt}")
                s0 = smp.tile([P, 1], f32, tag="s0", name=f"s0_{rep}_{i}_{nt}")
                s1 = smp.tile([P, 1], f32, tag="s1", name=f"s1_{rep}_{i}_{nt}")
                nc.scalar.activation(att[0:nsz, 0:288], eps[0][0:nsz, :],
                                     Act.Exp, bias=nm[0:nsz, :],
                                     accum_out=s0[0:nsz, :])
                nc.scalar.activation(att[0:nsz, 288:576], eps[1][0:nsz, :],
                                     Act.Exp, bias=nm[0:nsz, :],
                                     accum_out=s1[0:nsz, :])
                ssum = smp.tile([P, 1], f32, tag="ss", name=f"ss_{rep}_{i}_{nt}")
                rec = smp.tile([P, 1], f32, tag="rec", name=f"rec_{rep}_{i}_{nt}")
                nc.vector.tensor_add(ssum[0:nsz, :], s0[0:nsz, :], s1[0:nsz, :])
                nc.vector.reciprocal(rec[0:nsz, :], ssum[0:nsz, :])
                nc.vector.tensor_scalar_mul(att[0:nsz, :], att[0:nsz, :],
                                            rec[0:nsz, 0:1])
                for ctb, (coff, csz) in enumerate(NT):
                    pt = psT.tile([P, P], bf16, tag="pt", name=f"pt{rep}_{i}_{nt}_{ctb}")
                    nc.tensor.transpose(pt[0:csz, 0:nsz],
                                        att[0:nsz, coff:coff + csz],
                                        identb[0:nsz, 0:nsz])
                    nc.vector.tensor_copy(
                        attnT_sb[0:csz, i, ctb, noff:noff + nsz],
                        pt[0:csz, 0:nsz])

    # ---- Phase D: out_i^T, combined ----
    with tc.tile_pool(name=f"xs{rep}", bufs=3) as xsp, \
         tc.tile_pool(name=f"os{rep}", bufs=2) as osp, \
         tc.tile_pool(name=f"stg{rep}", bufs=2) as stgp, \
         tc.tile_pool(name=f"psD{rep}", bufs=4, space="PSUM") as psD, \
         tc.tile_pool(name=f"psDT{rep}", bufs=2, space="PSUM") as psDT:
        stage = {}
        for dt in range(CT):
            doff = dt * P
            outs = []
            for i in range(3):
                ot = osp.tile([P, HW], f32, tag=f"os{i}", name=f"os{rep}_{i}_{dt}")
                for (noff, nsz) in NCH:
                    ps = psD.tile([P, 288], f32, tag="dps", name=f"dps{rep}_{i}_{dt}_{noff}")
                    for tt, (toff, tsz) in enumerate(NT):
                        nc.tensor.matmul(
                            ps[:, :], lhsT=v_sb[0:tsz, tt, doff:doff + P],
                            rhs=attnT_sb[0:tsz, i, tt, noff:noff + nsz],
                            start=(tt == 0), stop=(tt == 4))
                    nc.scalar.activation(ot[:, noff:noff + nsz], ps[:, :],
                                         Act.Copy)
                nc.sync.dma_start(out=aout_d[i, doff:doff + P, :], in_=ot)
                outs.append(ot)
            xs = xsp.tile([P, HW], f32, tag="xs", name=f"xs{rep}_{dt}")
            nc.sync.dma_start(out=xs, in_=X32_d[doff:doff + P, :])
            u = osp.tile([P, HW], f32, tag="u", name=f"u{rep}_{dt}")
            nc.gpsimd.tensor_add(u, outs[0], outs[1])
            u2 = osp.tile([P, HW], f32, tag="u2", name=f"u2_{rep}_{dt}")
            nc.gpsimd.tensor_add(u2, u, outs[2])
            cmb = osp.tile([P, HW], f32, tag="cmb", name=f"cmb{rep}_{dt}")
            nc.vector.scalar_tensor_tensor(
                cmb, u2, gam_sb[:, 0:1], xs, op0=Alu.mult, op1=Alu.add)
            grp = dt % 4
            if grp == 0:
                stage = {nt: stgp.tile([P, 512], f32, tag=f"st{nt}",
                                       name=f"st{rep}_{nt}_{dt}")
                         for nt in range(5)}
            for nt, (noff, nsz) in enumerate(NT):
                pt = psDT.tile([P, P], f32, tag="pdt", name=f"pdt{rep}_{dt}_{nt}")
                nc.tensor.transpose(pt[0:nsz, 0:P], cmb[:, noff:noff + nsz],
                                    identf[:, :])
                nc.vector.tensor_copy(stage[nt][0:nsz, grp * P:(grp + 1) * P],
                                      pt[0:nsz, 0:P])
            if grp == 3:
                for nt, (noff, nsz) in enumerate(NT):
                    nc.sync.dma_start(
                        out=comb_d[noff:noff + nsz, (dt - 3) * P:(dt + 1) * P],
                        in_=stage[nt][0:nsz, :])


def run_spmd(nc, in_maps, time_iters=0):
    """Execute on the 8 axon cores via PJRT (no donation so the compiled fn can
    be re-invoked on device-resident buffers for timing)."""
    import jax
    import concourse.mybir as mybir
    from concourse import bass2jax
    from jax.sharding import Mesh, NamedSharding, PartitionSpec
    from jax.experimental.shard_map import shard_map

    bass2jax.install_neuronx_cc_hook()
    partition_name = nc.partition_id_tensor.name if nc.partition_id_tensor else None
    in_names, out_names, out_avals = [], [], []
    for alloc in nc.m.functions[0].allocations:
        if not isinstance(alloc, mybir.MemoryLocationSet):
            continue
        name = alloc.memorylocations[0].name
        if alloc.kind == "ExternalInput":
            if name != partition_name:
                in_names.append(name)
        elif alloc.kind == "ExternalOutput":
            out_names.append(name)
            out_avals.append(jax.core.ShapedArray(tuple(alloc.tensor_shape),
                                                  mybir.dt.np(alloc.dtype)))
    n_params = len(in_names)
    all_in = in_names + out_names
    if partition_name is not None:
        all_in = all_in + [partition_name]

    def _body(*args):
        operands = list(args)
        if partition_name is not None:
            operands.append(bass2jax.partition_id_tensor())
        return tuple(bass2jax._bass_exec_p.bind(
            *operands, out_avals=tuple(out_avals), in_names=tuple(all_in),
            out_names=tuple(out_names), lowering_input_output_aliases=(),
            sim_require_finite=True, sim_require_nnan=True, nc=nc))

    devices = jax.devices()[:N_CORES]
    mesh = Mesh(np.asarray(devices), ("core",))
    nspec = (PartitionSpec("core"),)
    sharded = jax.jit(shard_map(_body, mesh=mesh,
                                in_specs=nspec * (n_params + len(out_names)),
                                out_specs=nspec * len(out_names), check_rep=False),
                      keep_unused=True)
    sh = NamedSharding(mesh, PartitionSpec("core"))
    dev_in = [jax.device_put(
        np.concatenate([np.asarray(in_maps[c][nm]) for c in range(N_CORES)], axis=0),
        sh) for nm in in_names]
    dev_zero = [jax.device_put(
        np.zeros((N_CORES * a.shape[0], *a.shape[1:]), a.dtype), sh)
        for a in out_avals]
    out_arrs = sharded(*dev_in, *dev_zero)
    jax.block_until_ready(out_arrs)
    per_iter_ns = None
    if time_iters:
        import time
        t0 = time.perf_counter()
        for _ in range(time_iters):
            out_arrs = sharded(*dev_in, *dev_zero)
        jax.block_until_ready(out_arrs)
        per_iter_ns = (time.perf_counter() - t0) / time_iters * 1e9
    results = [{name: np.asarray(out_arrs[i]).reshape(N_CORES, *out_avals[i].shape)[c]
                for i, name in enumerate(out_names)} for c in range(N_CORES)]
    return results, per_iter_ns


def make_in_maps(x, Wq, bq, Wk, bk, Wv, bv, Wc, bc, gamma):
    import ml_dtypes
    bfdt = ml_dtypes.bfloat16
    f = lambda a: np.ascontiguousarray(np.asarray(a, dtype=np.float32))

    def split(a):
        a = f(a)
        hi = a.astype(bfdt)
        lo = (a - hi.astype(np.float32)).astype(bfdt)
        return np.ascontiguousarray(hi), np.ascontiguousarray(lo)

    WqTh, WqTl = split(np.asarray(Wq, np.float32).T)
    WkTh, WkTl = split(np.asarray(Wk, np.float32).T)
    WvT = np.ascontiguousarray(f(np.asarray(Wv, np.float32).T).astype(bfdt))
    WcTh, WcTl = split(np.transpose(np.asarray(Wc, np.float32), (0, 2, 1)))
    shared = {"WqTh": WqTh, "WqTl": WqTl, "WkTh": WkTh, "WkTl": WkTl,
              "WvT": WvT, "WcTh": WcTh, "WcTl": WcTl,
              "bq": f(bq), "bc": f(bc),
              "bkb": f(bk).astype(bfdt), "bvb": f(bv).astype(bfdt),
              "gamma": f(gamma), "ones": np.ones(P, dtype=bfdt),
              "identf": np.eye(P, dtype=np.float32),
              "identb": np.eye(P, dtype=bfdt)}
    in_maps = []
    for b in range(B):
        X32 = f(np.asarray(x[b], np.float32).reshape(HW, C).T)
        Xh, Xl = split(X32)
        in_maps.append({"X32": X32, "Xh": Xh, "Xl": Xl, **shared})
    return in_maps


def assemble(results):
    combined = np.stack([results[b]["combined_out"].reshape(H, Wd, C)
                         for b in range(B)], axis=0)
    attn_outs = np.stack([results[b]["attn_outs_out"].reshape(3, C, H, Wd)
                          for b in range(B)], axis=1)
    return combined, attn_outs


def kernel(x, Wq, bq, Wk, bk, Wv, bv, Wc, bc, gamma):
    nc = build_nc()
    in_maps = make_in_maps(x, Wq, bq, Wk, bk, Wv, bv, Wc, bc, gamma)
    results, _ = run_spmd(nc, in_maps)
    return assemble(results)
